# revision 36
# baseline (speedup 1.0000x reference)
"""Trainium2 Bass kernel: 3x3 same-padding Conv2D, NCHW.

Input  (16, 64, 128, 128) f32, weights (128, 64, 3, 3) OIHW, bias (128,).
Output (16, 128, 128, 128) f32.  HW exec ~87-90us on 8 NeuronCores.

Strategy: data-parallel over batch — 2 images per NeuronCore on 8 cores.
Per core the conv runs as accumulated TensorEngine matmuls over
(C_in x tap) contractions.  The PE streams 512-col matmul slots at
~216ns (fp16, 1 col/cycle) and this kernel keeps it gapless from
~10us to ~80us; the first ~17us of wall time runs at ~52% PE rate
(hardware power-ramp throttle, time-based, unavoidable).

  - The host pre-builds a padded dual fp16 layout per image,
    [128, 130*130]: partitions 0-63 (copy A) hold the zero-padded image
    shifted down one row (A[r] = padded row r-1), partitions 64-127
    (copy B) the padded rows directly (B[r] = padded row r).
  - DMA model (measured): all queues share a pool of 16 engines,
    ~26GB/s each (~416GB/s aggregate); a single queue sustains only
    ~150-200GB/s and processes its DMAs near-serially with ~1us fixed
    cost each.  Total traffic (8.65MB in + 16.8MB out) makes this a
    ridge-regime problem — the schedule matters more than any single
    transfer:
      * input chunk edges are 4096B-aligned (partial packets halve ring
        efficiency), with four 1152B lead-in chunks so supergroup 0's
        banks unblock one by one;
      * image-0 chunks split across the sync and scalar HW rings (the
        gpsimd SW ring has multi-us latency, carries only the bias);
      * image-1 chunks are issued by the scalar engine between
        supergroup evacuations — compute-paced so input never floods
        the engine pool and starves stores (store backlog stalls the
        scalar engine on DMA-semaphore reuse, which delays PSUM
        evacuation and gaps the PE);
      * stores: two 512KB half-stores per supergroup on the scalar and
        sync rings (sync joins after image-0 input drains off it).
  - Supergroups of 16 output rows use four PSUM banks (bank j holds
    rows h+4j..h+4j+3; 4*128 = 512 f32 = one bank), two supergroups in
    flight (8 banks).  Per supergroup:
      K=128 phase: per kw, one w1[kw] load feeds 4 matmuls (taps
      (kh=0,kw) on A + (kh=1,kw) on B fused in one K=128 contraction).
      K=64 phase: the (kh=2,kw) taps are issued as ADJACENT matmuls on
      disjoint partition halves (banks 0/2 from A on partitions 0-63,
      banks 1/3 from B on 64-127) so the PE runs pairs concurrently.
    => 18 matmul slots per 16 rows, the K=128-packing ideal.  (32-row
    supergroups with single-buffered banks measured WORSE: ~1.3us
    evacuation-wait stalls per supergroup boundary.)
  - Epilogue: ScalarE and VectorE each bias-add two banks into a shared
    [128, 2048] tile.  Image 1's last 16 rows run as two 8-row mini
    supergroups so the final evac+store unit is 2x256KB on parallel
    queues — shortens the post-compute tail by ~1-2us.

Operands are fp16 (cast host-side; 1 PE cycle/row + FWL weight loads,
rel err ~2.8e-4).  "f32r" mode (TF32-like) is kept as a fallback.
fp8 would double PE rate but rel err ~5e-2 fails the 2e-2 gate (even
with one-sided residual splits); Winograd F(2,3) is transform-bound on
the vector/scalar engines.  Every instruction may carry at most ONE
semaphore wait on this toolchain — bacc.Bacc's compile() pipeline
(generate_event_semaphores) enforces that, which is why this builds a
Bacc, not a raw bass.Bass.
"""

import sys

if "/opt/trn_rl_repo" not in sys.path:
    sys.path.insert(0, "/opt/trn_rl_repo")

import numpy as np

N_CORES = 8
IMGS_PER_CORE = 2
H = 128
W = 128
CIN = 64
COUT = 128
WPAD = W + 2  # 130: one zero column each side
HPAD = H + 2  # 130 rows (pad row above and below)
ROWS_PER_BANK = 4  # 4*128 = 512 free elements = one PSUM bank
SG_ROWS = 16  # supergroup: 4 PSUM banks

# "f32r": fp32 storage, TF32-like matmul (rel err ~2e-4, ~3 PE cycles/row)
# "f16": fp16 operands via host-side cast (rel err ~3e-4, 1 PE cycle/row)
DTYPE_MODE = "f16"

# Input chunk edges (elements of the flat [128, 130*130] image).  All
# interior edges are multiples of 2048 elems = 4096B, so every DMA
# descriptor row is whole 4096B packets.  Four small lead-in chunks
# (576 elems = 1152B = 1 packet) cover the first supergroup's rows
# (needs up to elem 17*130 = 2210 <= 2304) with minimum latency —
# alternating between two HW rings, the first matmul's data (<=517)
# lands ~1.4us sooner than with one 768-elem lead-in per ring.
X_EDGES = [0, 576, 1152, 1728, 2304] + list(range(4352, 16641, 2048)) + [16900]
# bank j of supergroup 0 needs flat elements < (4j+6)*130; lead-in
# chunk j (576 elems) covers bank j's reads, so banks unblock one by
# one as the two rings race the PE through supergroup 0.

_cache = {}


def _build_nc(mode=None):
    import concourse.mybir as mybir
    from concourse import bacc
    from concourse.tile import TileContext

    mode = mode or DTYPE_MODE
    f32 = mybir.dt.float32
    f32r = mybir.dt.float32r
    cdt = {"f32r": f32r, "f16": mybir.dt.float16}[mode]

    nc = bacc.Bacc(target_bir_lowering=False)
    x_d = nc.dram_tensor(
        "x", [IMGS_PER_CORE, 128, HPAD * WPAD], cdt, kind="ExternalInput"
    )
    # packed weights+bias:
    #   cols 0..383   : w1[t*64+ci, kw*128+co] = W[co, ci, t, kw], taps kh=t in {0,1}
    #   cols 384..767 : w2[ci, kw*128+co] = W[co, ci, 2, kw] (dup'd on rows 64-127)
    wb_d = nc.dram_tensor("wb", [128, 6 * COUT], cdt, kind="ExternalInput")
    b_d = nc.dram_tensor("b", [COUT, 1], f32, kind="ExternalInput")
    out_d = nc.dram_tensor(
        "out", [IMGS_PER_CORE, COUT, H, W], f32, kind="ExternalOutput"
    )

    with TileContext(nc) as tc:
        with (
            tc.tile_pool(name="wpool", bufs=1) as wpool,
            tc.tile_pool(name="xpool", bufs=2) as xpool,
            tc.tile_pool(name="opool", bufs=6) as opool,
            tc.tile_pool(name="pspool", bufs=2, space="PSUM") as pspool,
        ):
            wb_sb = wpool.tile([128, 6 * COUT], cdt)
            # three-way weight load, by when each piece gates compute:
            # w1[kw=0] (32KB) heads sync (gates the first matmul);
            # w1[kw=1,2] (64KB) heads scalar (gates matmul 5, ~+1.7us
            # throttled) without delaying input chunk c1 much; w2
            # (kh=2 taps, 96KB) rides the high-latency gpsimd ring
            # (first needed at supergroup 0's pair phase, ~+5us)
            nc.sync.dma_start(out=wb_sb[:, 0:COUT], in_=wb_d[:, 0:COUT])
            nc.scalar.dma_start(
                out=wb_sb[:, COUT : 3 * COUT], in_=wb_d[:, COUT : 3 * COUT]
            )
            nc.gpsimd.dma_start(
                out=wb_sb[:, 3 * COUT :], in_=wb_d[:, 3 * COUT :]
            )
            w1_sb = wb_sb[:, 0 : 3 * COUT]
            w2_sb = wb_sb[0:CIN, 3 * COUT : 6 * COUT]
            w2b_sb = wb_sb[CIN:128, 3 * COUT : 6 * COUT]
            b_f32 = wpool.tile([COUT, 1], f32)
            # bias on the gpsimd ring: keeps the sync ring free for the
            # first input chunks (bias isn't needed until the first
            # supergroup's epilogue, ~10us in)
            nc.gpsimd.dma_start(out=b_f32[:], in_=b_d[:])
            b_sb = b_f32[:]

            chunks = list(zip(X_EDGES[:-1], X_EDGES[1:]))
            n_chunks = len(chunks)
            # image-1 chunk issues are interleaved into image-0's
            # supergroup loop on the scalar engine, so they are paced by
            # compute progress (~1-2 chunks per 3.9us supergroup) instead
            # of flooding the shared DMA-engine pool and starving stores.
            n_sgs = H // SG_ROWS
            img1_sched = [[] for _ in range(n_sgs)]
            ci = 0
            for s in range(n_sgs):
                take = 2 if s < n_chunks - n_sgs else 1
                img1_sched[s] = list(range(ci, min(ci + take, n_chunks)))
                ci += take

            X0 = xpool.tile([128, HPAD * WPAD], cdt)
            X1 = xpool.tile([128, HPAD * WPAD], cdt)
            X_tiles = [X0, X1]
            # image-0 chunks split across the two fast HWDGE rings (sync
            # leads with c0 since scalar's queue head is the weight
            # tile); the gpsimd SW ring has multi-us per-DMA latency and
            # is only used for the tiny bias load.
            for k, (e0, e1) in enumerate(chunks):
                eng = nc.sync if k % 2 == 0 else nc.scalar
                eng.dma_start(out=X0[:, e0:e1], in_=x_d[0, :, e0:e1])

            unit = 0  # global supergroup counter (phase-order parity)
            for img in range(IMGS_PER_CORE):
                X = X_tiles[img]
                X3 = X.rearrange("p (r c) -> p r c", c=WPAD)

                # image 1's final 16 rows run as two 8-row mini
                # supergroups (2 banks each): the very last evac+store
                # unit is then 2x256KB on parallel queues instead of one
                # 1MB group — the post-compute drain (~150GB/s per queue,
                # serialized per DMA) shortens by ~2.5us for ~0.3us of
                # extra phase-boundary cost.
                if img < IMGS_PER_CORE - 1:
                    sg_list = [(h, 4) for h in range(0, H, SG_ROWS)]
                else:
                    sg_list = [(h, 4) for h in range(0, H - SG_ROWS, SG_ROWS)]
                    sg_list += [(H - SG_ROWS, 2), (H - SG_ROWS + 8, 2)]

                for s_idx, (h, nb) in enumerate(sg_list):
                    ps = [
                        pspool.tile(
                            [COUT, ROWS_PER_BANK * W],
                            f32,
                            tag=f"ps{j}",
                            name=f"ps{j}",
                        )
                        for j in range(nb)
                    ]
                    # Two phases per supergroup:
                    #   K=128: taps (kh=0,kw) on A + (kh=1,kw) on B fused
                    #   in one contraction, one weight tile per kw for
                    #   all banks.
                    #   K=64: the (kh=2,kw) taps as ADJACENT matmuls on
                    #   disjoint partition halves (A-half -> even banks,
                    #   B-half -> odd banks) so the PE runs pairs
                    #   concurrently.
                    # The phase ORDER alternates by supergroup parity, so
                    # consecutive supergroups meet in the same PE tile
                    # size — each 128x128 <-> 64x128 LDWEIGHTS switch
                    # costs ~95ns of exposed load, and alternation halves
                    # the switch count.
                    def k128_phase(first):
                        for kw in range(3):
                            for j in range(nb):
                                r = h + ROWS_PER_BANK * j
                                nc.tensor.matmul(
                                    ps[j][:],
                                    w1_sb[:, kw * COUT : (kw + 1) * COUT],
                                    X3[:, r : r + ROWS_PER_BANK, kw : kw + W],
                                    start=(first and kw == 0),
                                    stop=(not first and kw == 2),
                                )

                    def k64_phase(first):
                        for kw in range(3):
                            for j in range(0, nb, 2):
                                rA = h + ROWS_PER_BANK * j + 2
                                rB = h + ROWS_PER_BANK * (j + 1) + 1
                                nc.tensor.matmul(
                                    ps[j][:],
                                    w2_sb[:, kw * COUT : (kw + 1) * COUT],
                                    X3[
                                        0:CIN, rA : rA + ROWS_PER_BANK, kw : kw + W
                                    ],
                                    start=(first and kw == 0),
                                    stop=(not first and kw == 2),
                                )
                                nc.tensor.matmul(
                                    ps[j + 1][:],
                                    w2b_sb[:, kw * COUT : (kw + 1) * COUT],
                                    X3[
                                        CIN:128,
                                        rB : rB + ROWS_PER_BANK,
                                        kw : kw + W,
                                    ],
                                    start=(first and kw == 0),
                                    stop=(not first and kw == 2),
                                )

                    if unit % 2 == 0:
                        k128_phase(True)
                        k64_phase(False)
                    else:
                        k64_phase(True)
                        k128_phase(False)
                    unit += 1
                    # bias-add while evacuating PSUM into one 16-row tile;
                    # ScalarE takes banks 0,2 and VectorE banks 1,3.  Two
                    # 512KB store DMAs per supergroup on the scalar and
                    # vector HWDGE rings (stores never queue behind input
                    # chunks on sync/gpsimd).
                    FB = ROWS_PER_BANK * W  # 512
                    ob = opool.tile([COUT, nb * FB], f32)
                    for j in range(0, nb, 2):
                        nc.scalar.add(
                            ob[:, j * FB : (j + 1) * FB], ps[j][:], b_sb
                        )
                        nc.vector.tensor_scalar_add(
                            ob[:, (j + 1) * FB : (j + 2) * FB], ps[j + 1][:], b_sb
                        )
                    ob3 = ob.rearrange("p (r c) -> p r c", c=W)
                    if nb == 4:
                        # two 512KB half-stores per supergroup —
                        # per-queue DMA processing is serialized with
                        # ~1us fixed cost per DMA, so fewer/larger stores
                        # drain faster than fine-grained splits
                        # (measured).  Banks 0/1 stop one pair-slot
                        # early, so the first half-store issues ~0.4us
                        # after the supergroup's last matmul.
                        nc.scalar.dma_start(
                            out=out_d[img, :, h : h + 8, :], in_=ob3[:, 0:8]
                        )
                        # second half-store on the sync ring once image
                        # 0's input chunks have drained off it (~21us);
                        # before that, both halves go on scalar
                        eng2 = nc.scalar if (img == 0 and s_idx < 2) else nc.sync
                        eng2.dma_start(
                            out=out_d[img, :, h + 8 : h + 16, :], in_=ob3[:, 8:16]
                        )
                    else:
                        # 8-row mini supergroup (tail of image 1): one
                        # 256KB store per bank on parallel queues
                        nc.scalar.dma_start(
                            out=out_d[img, :, h : h + 4, :], in_=ob3[:, 0:4]
                        )
                        nc.sync.dma_start(
                            out=out_d[img, :, h + 4 : h + 8, :], in_=ob3[:, 4:8]
                        )
                    if img == 0:
                        # compute-paced image-1 input: issued by the
                        # sync engine after this supergroup's stores.
                        # The sync engine's half-store issues block on
                        # evacuation semaphores, so chunks queued after
                        # them are paced by compute without loading the
                        # busier scalar engine (2 ACTIVATEs + store
                        # issues every supergroup).
                        for k in img1_sched[s_idx]:
                            e0, e1 = chunks[k]
                            nc.sync.dma_start(
                                out=X1[:, e0:e1], in_=x_d[1, :, e0:e1]
                            )
    nc.compile()
    return nc


def _get_nc(mode=None):
    mode = mode or DTYPE_MODE
    if mode not in _cache:
        _cache[mode] = _build_nc(mode)
    return _cache[mode]


def _make_dual(images):
    """images: [n, 64, 128, 128] -> [n, 128, HPAD*WPAD] dual padded layout."""
    n = images.shape[0]
    zp = np.zeros((n, CIN, HPAD, WPAD), dtype=np.float32)
    zp[:, :, 1 : H + 1, 1 : W + 1] = images  # padded rows 0..129
    dual = np.empty((n, 128, HPAD, WPAD), dtype=np.float32)
    dual[:, 0:CIN] = zp  # A[r] = padded row r-1 shape-wise (row r of zp)
    dual[:, CIN:128, 0 : HPAD - 1] = zp[:, :, 1:HPAD]  # B[r] = padded row r
    dual[:, CIN:128, HPAD - 1] = 0.0  # B row 129 unread
    return np.ascontiguousarray(dual.reshape(n, 128, HPAD * WPAD))


def _prepare_in_maps(input_tensor, weights, bias, mode=None):
    mode = mode or DTYPE_MODE
    hdt = np.float32 if mode == "f32r" else np.float16
    input_tensor = np.asarray(input_tensor, dtype=np.float32)
    weights = np.asarray(weights, dtype=np.float32)
    bias = np.asarray(bias, dtype=np.float32)
    wb = np.zeros((128, 6 * COUT), dtype=np.float32)
    # [co, ci, kh, kw] -> w1[t*64+ci, kw*128+co], w2[ci, kw*128+co]
    wb[:, 0 : 3 * COUT] = (
        weights[:, :, 0:2, :].transpose(2, 1, 3, 0).reshape(128, 3 * COUT)
    )
    w2 = weights[:, :, 2, :].transpose(1, 2, 0).reshape(CIN, 3 * COUT)
    wb[0:CIN, 3 * COUT : 6 * COUT] = w2
    wb[CIN:128, 3 * COUT : 6 * COUT] = w2  # duplicate for partition-64 row tiles
    wb = np.ascontiguousarray(wb.astype(hdt))
    b = np.ascontiguousarray(bias.reshape(COUT, 1))
    in_maps = []
    for c in range(N_CORES):
        shard = _make_dual(
            input_tensor[c * IMGS_PER_CORE : (c + 1) * IMGS_PER_CORE]
        ).astype(hdt)
        in_maps.append({"x": shard, "wb": wb, "b": b})
    return in_maps


def _gather(results):
    return np.concatenate([results[c]["out"] for c in range(N_CORES)], axis=0)


def kernel(input_tensor, weights, bias):
    from concourse.bass_utils import run_bass_kernel_spmd

    nc = _get_nc()
    in_maps = _prepare_in_maps(input_tensor, weights, bias)
    res = run_bass_kernel_spmd(nc, in_maps, core_ids=list(range(N_CORES)))
    return _gather(res.results)
